# revision 10
# baseline (speedup 1.0000x reference)
"""Karras optimal denoiser (kNN softmax over training set) on 8 trn2 cores.

out[b] = sum_n softmax_n(-0.5*||x_b - y_n||^2 / sigma_b^2) * y_n

Softmax row-constant terms cancel, so per-core we compute
  U[b,n] = x_b . y_n - 0.5*||y_n||^2        (one GEMM, augmented row)
  L      = inv_var[b] * U                   (folded into exp's scale)
and the flash-style partials (m_b, s_b, acc_b) over the core's N-shard,
merged on the host with a logsumexp merge.
"""

import math
import sys

sys.path.insert(0, "/opt/trn_rl_repo")

import numpy as np

B = 64
CC, HH, WW = 3, 32, 32
D = CC * HH * WW  # 3072
N = 50000
NCORES = 8

NP = 6656          # padded per-core shard (52*128 = 13*512)
GW = 256           # phase-1 group width (psum free dim)
NGROUPS = NP // GW     # 13
NK = NP // 128         # 52 chunks for phase 3
DP = 3200          # 3072 data rows + y2 row + zero pad = 25*128
KCH = DP // 128        # 25
DG = D // 512          # 6 psum banks for the output
Y2_PAD = 1.0e8     # padded columns get huge ||y||^2 -> exp underflows to 0

_PROGRAM_CACHE: dict = {}


def _build_program():
    import concourse.bass as bass
    import concourse.bacc as bacc
    import concourse.mybir as mybir
    import concourse.tile as tile
    from concourse.bass import ts

    f32 = mybir.dt.float32
    f32r = mybir.dt.float32r
    Exp = mybir.ActivationFunctionType.Exp
    AX = mybir.AxisListType.X
    mx = mybir.AluOpType.max
    mult = mybir.AluOpType.mult

    nc = bacc.Bacc()
    xt_d = nc.declare_dram_parameter("xt", [DP, B], f32, isOutput=False)
    iv_d = nc.declare_dram_parameter("iv", [B], f32, isOutput=False)
    yt_d = nc.declare_dram_parameter("yt", [DP, NP], f32, isOutput=False)
    yn_d = nc.declare_dram_parameter("yn", [NP, D], f32r, isOutput=False)
    id_d = nc.declare_dram_parameter("ident", [B, B], f32r, isOutput=False)
    out_d = nc.declare_dram_parameter("out", [B, D + 2], f32, isOutput=True)

    yt_v = yt_d.rearrange("(k p) n -> p k n", p=128)   # [128, 25, NP]
    xt_v = xt_d.rearrange("(k p) m -> p k m", p=128)   # [128, 25, B]

    with tile.TileContext(nc) as tc:
        with (
            tc.tile_pool(name="const", bufs=1) as constp,
            tc.tile_pool(name="yt", bufs=2) as ytp,
            tc.tile_pool(name="yn", bufs=3) as ynp,
            tc.tile_pool(name="work", bufs=1) as workp,
        ):
            xt_sb = constp.tile([128, KCH, B], f32, tag="xt")
            nc.sync.dma_start(out=xt_sb[:], in_=xt_v[:])
            iv_sb = constp.tile([B, 1], f32, tag="iv")
            nc.sync.dma_start(out=iv_sb[:, 0], in_=iv_d[:])
            ident = constp.tile([B, B], f32r, tag="ident")
            nc.sync.dma_start(out=ident[:], in_=id_d[:])

            l_sb = workp.tile([B, NGROUPS, GW], f32, tag="l")
            w_sb = workp.tile([B, NGROUPS, GW], f32r, tag="w")
            wt_sb = workp.tile([128, NK, B], f32r, tag="wt")
            mx_parts = workp.tile([B, NGROUPS], f32, tag="mxp")
            m_u = workp.tile([B, 1], f32, tag="mu")
            negb = workp.tile([B, 1], f32, tag="negb")
            m_out = workp.tile([B, 1], f32, tag="mout")
            s_sb = workp.tile([B, 1], f32, tag="s")
            acc_sb = workp.tile([B, D], f32, tag="accsb")

            # ---- phase 1: U = x.y - 0.5*y2  (fp32 GEMM), logits to SBUF ----
            with tc.tile_pool(name="psum1", bufs=2, space="PSUM") as psum1:
                for j in range(NGROUPS):
                    yt_t = ytp.tile([128, KCH, GW], f32, tag="ytt")
                    nc.sync.dma_start(
                        out=yt_t[:], in_=yt_v[:, :, ts(j, GW)]
                    )
                    l_ps = psum1.tile([B, GW], f32, tag="L")
                    for k in range(KCH):
                        nc.tensor.matmul(
                            l_ps[:],
                            xt_sb[:, k, :],
                            yt_t[:, k, :],
                            start=(k == 0),
                            stop=(k == KCH - 1),
                        )
                    nc.vector.tensor_reduce(
                        out=mx_parts[:, j : j + 1], in_=l_ps[:], axis=AX, op=mx
                    )
                    nc.any.tensor_copy(l_sb[:, j, :], l_ps[:])

                # ---- phase 2: softmax weights W = exp(iv*(U - mU)) ----
                nc.vector.tensor_reduce(
                    out=m_u[:], in_=mx_parts[:], axis=AX, op=mx
                )
                nc.vector.tensor_tensor(
                    out=negb[:], in0=m_u[:], in1=iv_sb[:], op=mult
                )
                nc.vector.tensor_scalar_mul(negb[:], negb[:], -1.0)
                nc.vector.tensor_scalar_mul(m_out[:], negb[:], -1.0)
                nc.scalar.activation(
                    out=w_sb[:],
                    in_=l_sb[:],
                    func=Exp,
                    bias=negb[:],
                    scale=iv_sb[:],
                    accum_out=s_sb[:],
                )

                # ---- phase 2b: transpose W -> WT tiles [128n, B] ----
                for kk in range(NK):
                    j, t = divmod(kk, GW // 128)
                    wt_ps = psum1.tile([128, B], f32r, tag="wtps")
                    nc.tensor.transpose(
                        wt_ps[:], w_sb[:, j, ts(t, 128)], ident[:]
                    )
                    nc.any.tensor_copy(wt_sb[:, kk, :], wt_ps[:])

            # ---- phase 3: acc = W @ Y  (f32r GEMM, 6 psum banks) ----
            with tc.tile_pool(name="psum2", bufs=1, space="PSUM") as psum2:
                accs = [
                    psum2.tile([B, 512], f32, tag=f"acc{g}", name=f"acc{g}")
                    for g in range(DG)
                ]
                for kk in range(NK):
                    yn_t = ynp.tile([128, D], f32r, tag="ynt")
                    nc.sync.dma_start(out=yn_t[:], in_=yn_d[ts(kk, 128), :])
                    for g in range(DG):
                        nc.tensor.matmul(
                            accs[g][:],
                            wt_sb[:, kk, :],
                            yn_t[:, ts(g, 512)],
                            start=(kk == 0),
                            stop=(kk == NK - 1),
                        )
                for g in range(DG):
                    nc.any.tensor_copy(acc_sb[:, ts(g, 512)], accs[g][:])
            nc.sync.dma_start(out=out_d[:, 0:D], in_=acc_sb[:])
            nc.sync.dma_start(out=out_d[:, D], in_=m_out[:, 0])
            nc.sync.dma_start(out=out_d[:, D + 1], in_=s_sb[:, 0])

    nc.compile()
    return nc


def _get_program():
    if "nc" not in _PROGRAM_CACHE:
        _PROGRAM_CACHE["nc"] = _build_program()
    return _PROGRAM_CACHE["nc"]


def _prep_inputs(x, sigma, Y):
    xf = np.ascontiguousarray(x.reshape(B, D)).astype(np.float32)
    Yf = np.ascontiguousarray(Y.reshape(N, D)).astype(np.float32)
    sigma = sigma.astype(np.float32)
    inv_var = (1.0 / (sigma * sigma)).astype(np.float32)

    xt = np.zeros((DP, B), dtype=np.float32)
    xt[:D, :] = xf.T
    xt[D, :] = -0.5

    y2 = np.einsum("nd,nd->n", Yf, Yf).astype(np.float32)

    per_core = []
    nsh = N // NCORES  # 6250
    for c in range(NCORES):
        sl = slice(c * nsh, (c + 1) * nsh)
        yt_c = np.zeros((DP, NP), dtype=np.float32)
        yt_c[:D, :nsh] = Yf[sl].T
        yt_c[D, :nsh] = y2[sl]
        yt_c[D, nsh:] = Y2_PAD
        yn_c = np.zeros((NP, D), dtype=np.float32)
        yn_c[:nsh] = Yf[sl]
        per_core.append(
            {"xt": xt, "iv": inv_var, "yt": yt_c, "yn": yn_c,
             "ident": np.eye(B, dtype=np.float32)}
        )
    return per_core


def _merge(results):
    # per-core outputs: out[:, :D]=acc, out[:, D]=m, out[:, D+1]=s
    ms = np.stack([r["out"][:, D] for r in results])       # [NCORES, B]
    ss = np.stack([r["out"][:, D + 1] for r in results])   # [NCORES, B]
    accs = np.stack([r["out"][:, :D] for r in results])    # [NCORES, B, D]
    m_glob = ms.max(axis=0)                                # [B]
    corr = np.exp(ms - m_glob[None, :])                    # [NCORES, B]
    s_tot = (ss * corr).sum(axis=0)                        # [B]
    acc_tot = np.einsum("cb,cbd->bd", corr, accs)          # [B, D]
    return acc_tot / s_tot[:, None]


def kernel(x, sigma, Y):
    from concourse.bass_utils import run_bass_kernel_spmd

    nc = _get_program()
    in_maps = _prep_inputs(np.asarray(x), np.asarray(sigma), np.asarray(Y))
    res = run_bass_kernel_spmd(nc, in_maps, list(range(NCORES)))
    out = _merge(res.results)
    return out.reshape(B, CC, HH, WW).astype(np.float32)


if __name__ == "__main__":
    rng = np.random.default_rng(0)
    x = rng.standard_normal((B, CC, HH, WW), dtype=np.float32)
    sigma = (rng.random(B, dtype=np.float32) * 1.9 + 0.1).astype(np.float32)
    Y = rng.standard_normal((N, CC, HH, WW), dtype=np.float32)
    out = kernel(x=x, sigma=sigma, Y=Y)
    print("out", out.shape, out.dtype, float(np.abs(out).mean()))
